# revision 12
# baseline (speedup 1.0000x reference)
"""AttentionXL Trainium2 kernel: 8-core tensor-parallel (2 heads/core).

Self-contained: hardcodes shapes from the problem spec.
  inputs:       (1024, 4, 1024) f32   cur_seq, bs, d
  full_input:   (2048, 4, 1024) f32   full_seq, bs, d
  pos_embedding:(2048, 1024)    f32
  u, v:         (16, 64)        f32   H, D
  Wkv (1024, 2*1024), Wq/Wr/Wo (1024, 1024), biases zero, mask all-False.

Per-core kernel (heads 2c, 2c+1):
  qT = Wq_c^T x_cur^T          [128, cs*bs]   (+u / +v fused on eviction)
  kT/vT = Wkv_c^T x_full^T     [128, fs*bs]
  rT = Wr_c^T pos^T            [128, fs]
  BD  = (q+v)^T r  per (b,h)   [i, j]  -> DRAM scratch with row pitch fs+1
  BDshifted^T <- DMA-transpose read of DRAM view with row stride fs, offset cs
     (reproduces the reference rel_shift flat-reinterpret exactly, incl. wrap)
  S^T = K^T(q+u) + I @ BDshifted^T   (PSUM accumulate)
  E^T = exp(S^T/8)  (ScalarE eviction, bf16)
  O^T[65] = [V | 1]^T E^T  (AV matmul; row 64 = softmax denominators)
  attn_vec = O^T[0:64] * (1/Z) ; Y^T_partial = Wo_c^T attn_vec   -> DRAM

Host: shard/cast/transpose inputs, run SPMD on 8 cores, sum partial Y.
"""

import os
from contextlib import ExitStack

import numpy as np
import ml_dtypes

import concourse.bass as bass
import concourse.bacc as bacc_mod
import concourse.mybir as mybir
import concourse.tile as tile
from concourse.bass_utils import run_bass_kernel_spmd
from concourse.masks import make_identity

BF16 = mybir.dt.bfloat16
F32 = mybir.dt.float32
NPBF16 = ml_dtypes.bfloat16

# Problem dims (full size)
CS, FS, BS, D_MODEL = 1024, 2048, 4, 1024
H, HD = 16, 64
N_CORES = 8
HPC = H // N_CORES          # heads per core = 2
DC = HPC * HD               # per-core model slice = 128


def build_core_kernel(cs=CS, fs=FS, bs=BS, d=D_MODEL, hpc=HPC, hd=HD):
    dc = hpc * hd
    assert dc == 128, "kernel assumes 128-partition per-core slice"
    nk = d // 128           # contraction chunks for projections
    TB = cs * bs            # query tokens (i-major, b-minor)
    TF = fs * bs            # kv tokens
    NI = cs // 128          # i tiles
    NJ = fs // 128          # j tiles
    NTQ = TB // 512         # 512-wide chunks of query tokens
    NTF = TF // 512
    NTP = fs // 512
    NIC = cs // 512         # i chunks (free dim of S^T)
    scale = 1.0 / (hd ** 0.5)

    nc = bacc_mod.Bacc(None, target_bir_lowering=False, debug=False)

    xcurT = nc.dram_tensor("xcurT", [d, TB], BF16, kind="ExternalInput")
    xfullT = nc.dram_tensor("xfullT", [d, TF], BF16, kind="ExternalInput")
    posT = nc.dram_tensor("posT", [d, fs], BF16, kind="ExternalInput")
    wq_d = nc.dram_tensor("wq", [d, dc], BF16, kind="ExternalInput")
    wk_d = nc.dram_tensor("wk", [d, dc], BF16, kind="ExternalInput")
    wv_d = nc.dram_tensor("wv", [d, dc], BF16, kind="ExternalInput")
    wr_d = nc.dram_tensor("wr", [d, dc], BF16, kind="ExternalInput")
    wo_d = nc.dram_tensor("wo", [dc, d], BF16, kind="ExternalInput")
    u_d = nc.dram_tensor("u", [dc, 1], F32, kind="ExternalInput")
    v_d = nc.dram_tensor("v", [dc, 1], F32, kind="ExternalInput")
    y_d = nc.dram_tensor("y", [bs, d, cs], BF16, kind="ExternalOutput")

    # DRAM scratch for the rel-shift pitch trick: one buffer per (b-parity, h).
    # Layout: [cs rows, fs+1 cols]; col 0 stays zero (the padded zero column).
    p2 = [
        nc.dram_tensor(f"p2_{i}", [cs * (fs + 1)], BF16)
        for i in range(4)
    ]

    with tile.TileContext(nc) as tc, ExitStack() as ctx:
        const = ctx.enter_context(tc.tile_pool(name="const", bufs=1))
        persist = ctx.enter_context(tc.tile_pool(name="persist", bufs=1))
        xs = ctx.enter_context(tc.tile_pool(name="xs", bufs=4))
        bdst = ctx.enter_context(tc.tile_pool(name="bdst", bufs=4))
        bdev = ctx.enter_context(tc.tile_pool(name="bdev", bufs=4))
        ea = ctx.enter_context(tc.tile_pool(name="ea", bufs=NJ + 4))
        vxp = ctx.enter_context(tc.tile_pool(name="vxp", bufs=NJ + 4))
        onrm = ctx.enter_context(tc.tile_pool(name="onrm", bufs=2))
        yout = ctx.enter_context(tc.tile_pool(name="yout", bufs=3))
        psA = ctx.enter_context(tc.tile_pool(name="psA", bufs=2, space="PSUM"))
        psB = ctx.enter_context(tc.tile_pool(name="psB", bufs=2, space="PSUM"))
        psO = ctx.enter_context(tc.tile_pool(name="psO", bufs=2, space="PSUM"))
        psX = ctx.enter_context(tc.tile_pool(name="psX", bufs=1, space="PSUM"))

        # ---- constants / weights in SBUF ----
        ident = const.tile([128, 128], BF16)
        make_identity(nc, ident[:])

        def load_w(dram, nm):  # [d, dc] -> SBUF [128, nk*dc], chunk kk at cols kk*dc
            t = const.tile([128, nk * dc], BF16, name=nm, tag=nm)
            src = bass.AP(
                tensor=dram, offset=0,
                ap=[[dc, 128], [128 * dc, nk], [1, dc]],
            )
            nc.sync.dma_start(out=t[:], in_=src)
            return t

        wq = load_w(wq_d, "wq_sb")
        wk = load_w(wk_d, "wk_sb")
        wv = load_w(wv_d, "wv_sb")
        wr = load_w(wr_d, "wr_sb")
        wo = const.tile([128, d], BF16)
        nc.sync.dma_start(out=wo[:], in_=wo_d[:, :])
        u_sb = const.tile([128, 1], F32)
        v_sb = const.tile([128, 1], F32)
        nc.sync.dma_start(out=u_sb[:], in_=u_d[:, :])
        nc.sync.dma_start(out=v_sb[:], in_=v_d[:, :])

        # zero column 0 of each p2 buffer
        zc = cs // 128
        zcol = const.tile([128, zc], BF16)
        nc.vector.memset(zcol[:], 0.0)
        for pb in p2:
            dst = bass.AP(tensor=pb, offset=0,
                          ap=[[fs + 1, 128], [(fs + 1) * 128, zc]])
            nc.sync.dma_start(out=dst, in_=zcol[:])

        # ---- persistent activations ----
        qTu = persist.tile([128, TB], BF16)   # (q + u)^T
        qTv = persist.tile([128, TB], BF16)   # (q + v)^T
        kT = persist.tile([128, TF], BF16)
        rT = persist.tile([128, fs], BF16)
        vTe = [persist.tile([65, TF], BF16, name=f"vTe{i}", tag=f"vTe{i}")
               for i in range(hpc)]  # v^T + ones row
        ofin = persist.tile([128, cs], BF16)  # normalized attn_vec^T for one b

        Ident = mybir.ActivationFunctionType.Identity
        Exp = mybir.ActivationFunctionType.Exp

        # ---- projections ----
        # q: accumulate over nk chunks; evict twice with +u and +v bias.
        for t0 in range(NTQ):
            ps = psA.tile([128, 512], F32, name="psa", tag="a")
            for kk in range(nk):
                xt = xs.tile([128, 512], BF16)
                nc.sync.dma_start(
                    out=xt[:], in_=xcurT[kk * 128:(kk + 1) * 128,
                                         t0 * 512:(t0 + 1) * 512])
                nc.tensor.matmul(ps[:], wq[:, kk * dc:(kk + 1) * dc], xt[:],
                                 start=(kk == 0), stop=(kk == nk - 1))
            sl = (slice(None), slice(t0 * 512, (t0 + 1) * 512))
            nc.scalar.activation(qTu[sl], ps[:], Ident, bias=u_sb[:, 0:1])
            nc.scalar.activation(qTv[sl], ps[:], Ident, bias=v_sb[:, 0:1])

        # k and v share the x_full chunk loads
        for t0 in range(NTF):
            psk = psA.tile([128, 512], F32, name="psk", tag="a")
            psv = psB.tile([128, 512], F32, name="psv", tag="b")
            for kk in range(nk):
                xt = xs.tile([128, 512], BF16)
                nc.sync.dma_start(
                    out=xt[:], in_=xfullT[kk * 128:(kk + 1) * 128,
                                          t0 * 512:(t0 + 1) * 512])
                nc.tensor.matmul(psk[:], wk[:, kk * dc:(kk + 1) * dc], xt[:],
                                 start=(kk == 0), stop=(kk == nk - 1))
                nc.tensor.matmul(psv[:], wv[:, kk * dc:(kk + 1) * dc], xt[:],
                                 start=(kk == 0), stop=(kk == nk - 1))
            sl = slice(t0 * 512, (t0 + 1) * 512)
            nc.scalar.copy(kT[:, sl], psk[:])
            for h in range(hpc):
                nc.scalar.copy(vTe[h][0:hd, sl], psv[h * hd:(h + 1) * hd, :])

        for t0 in range(NTP):
            ps = psA.tile([128, 512], F32, name="psa", tag="a")
            for kk in range(nk):
                xt = xs.tile([128, 512], BF16)
                nc.sync.dma_start(
                    out=xt[:], in_=posT[kk * 128:(kk + 1) * 128,
                                        t0 * 512:(t0 + 1) * 512])
                nc.tensor.matmul(ps[:], wr[:, kk * dc:(kk + 1) * dc], xt[:],
                                 start=(kk == 0), stop=(kk == nk - 1))
            nc.scalar.copy(rT[:, t0 * 512:(t0 + 1) * 512], ps[:])

        for h in range(hpc):
            nc.vector.memset(vTe[h][hd:hd + 1, :], 1.0)

        # token-indexed views (p, token) -> (p, seq, b)
        qTu3 = qTu[:].rearrange("p (i b) -> p i b", b=bs)
        qTv3 = qTv[:].rearrange("p (i b) -> p i b", b=bs)
        kT3 = kT[:].rearrange("p (j b) -> p j b", b=bs)
        vTe3 = [t[:].rearrange("p (j b) -> p j b", b=bs) for t in vTe]

        # ---- attention, one batch at a time ----
        for b in range(bs):
            for h in range(hpc):
                hs = slice(h * hd, (h + 1) * hd)
                pb = p2[(b % 2) * hpc + h]

                # BD raw = (q+v)^T_h r_h   in [i, j] layout -> DRAM pitch fs+1
                for it in range(NI):
                    for jc in range(NTP):
                        psbd = psB.tile([128, 512], F32, name="psbd", tag="b")
                        nc.tensor.matmul(
                            psbd[:],
                            qTv3[hs, it * 128:(it + 1) * 128, b],
                            rT[hs, jc * 512:(jc + 1) * 512],
                            start=True, stop=True)
                        st = bdev.tile([128, 512], BF16)
                        nc.vector.tensor_copy(st[:], psbd[:])
                        dst = bass.AP(
                            tensor=pb,
                            offset=(it * 128) * (fs + 1) + 1 + jc * 512,
                            ap=[[fs + 1, 128], [1, 512]])
                        nc.sync.dma_start(out=dst, in_=st[:])

                # shifted+transposed read back: view rows=cs stride fs, off cs
                bds = []
                for jt in range(NJ):
                    t = bdst.tile([128, cs], BF16)
                    src = bass.AP(tensor=pb, offset=cs + jt * 128,
                                  ap=[[fs, cs], [1, 128]])
                    nc.sync.dma_start(out=t[:], in_=src, transpose=True)
                    bds.append(t)

                # Vx = transpose([v | 1]) per j-tile
                vx = []
                for jt in range(NJ):
                    pvx = psX.tile([128, 65], BF16, name="pvx", tag="vx")
                    nc.tensor.transpose(
                        pvx[:], vTe3[h][0:65, jt * 128:(jt + 1) * 128, b],
                        ident[0:65, 0:65])
                    t = vxp.tile([128, 65], BF16)
                    nc.scalar.copy(t[:], pvx[:])
                    vx.append(t)

                # S^T tiles: content matmul + identity-add of shifted BD; exp
                eat = []
                for jt in range(NJ):
                    et = ea.tile([128, cs], BF16)
                    for ic in range(NIC):
                        psac = psA.tile([128, 512], F32, name="psac", tag="a")
                        nc.tensor.matmul(
                            psac[:],
                            kT3[hs, jt * 128:(jt + 1) * 128, b],
                            qTu3[hs, ic * 512:(ic + 1) * 512, b],
                            start=True, stop=False)
                        nc.tensor.matmul(
                            psac[:], ident[:, :],
                            bds[jt][:, ic * 512:(ic + 1) * 512],
                            start=False, stop=True)
                        nc.scalar.activation(
                            et[:, ic * 512:(ic + 1) * 512], psac[:],
                            Exp, scale=scale)
                    eat.append(et)

                # AV (+ softmax sums in row 64)
                for ic in range(NIC):
                    pso = psO.tile([65, 512], F32, name="pso", tag="o")
                    for jt in range(NJ):
                        nc.tensor.matmul(
                            pso[:], vx[jt][:], eat[jt][:, ic * 512:(ic + 1) * 512],
                            start=(jt == 0), stop=(jt == NJ - 1))
                    ov = onrm.tile([65, 512], F32)
                    nc.scalar.copy(ov[:], pso[:])
                    rc = onrm.tile([1, 512], F32)
                    nc.vector.reciprocal(rc[:], ov[hd:hd + 1, :])
                    rb = onrm.tile([hd, 512], F32)
                    nc.gpsimd.partition_broadcast(rb[:], rc[:])
                    nc.vector.tensor_mul(
                        ofin[h * hd:(h + 1) * hd, ic * 512:(ic + 1) * 512],
                        ov[0:hd, :], rb[:])

            # output projection for this batch
            for oc in range(d // 128):
                for ic in range(NIC):
                    psy = psX.tile([128, 512], F32, name="psy", tag="y")
                    nc.tensor.matmul(
                        psy[:], wo[:, oc * 128:(oc + 1) * 128],
                        ofin[:, ic * 512:(ic + 1) * 512],
                        start=True, stop=True)
                    yt = yout.tile([128, 512], BF16)
                    nc.scalar.copy(yt[:], psy[:])
                    nc.sync.dma_start(
                        out=y_d[b, oc * 128:(oc + 1) * 128,
                                ic * 512:(ic + 1) * 512],
                        in_=yt[:])

    nc.compile()
    return nc


_NC_CACHE = {}


def _get_nc(dims):
    if dims not in _NC_CACHE:
        _NC_CACHE[dims] = build_core_kernel(*dims)
    return _NC_CACHE[dims]


def make_in_maps(inputs, pos_embedding, full_input, u, v, Wkv, Wq, Wr, Wo,
                 cs=CS, fs=FS, bs=BS, d=D_MODEL, hpc=HPC, hd=HD,
                 n_cores=N_CORES):
    dc = hpc * hd
    xcurT = np.ascontiguousarray(
        np.asarray(inputs, np.float32).reshape(cs * bs, d).T).astype(NPBF16)
    xfullT = np.ascontiguousarray(
        np.asarray(full_input, np.float32).reshape(fs * bs, d).T).astype(NPBF16)
    posT = np.ascontiguousarray(
        np.asarray(pos_embedding, np.float32).T).astype(NPBF16)
    Wkv = np.asarray(Wkv, np.float32)
    Wq = np.asarray(Wq, np.float32)
    Wr = np.asarray(Wr, np.float32)
    Wo = np.asarray(Wo, np.float32)
    u = np.asarray(u, np.float32)
    v = np.asarray(v, np.float32)

    in_maps = []
    for c in range(n_cores):
        cols = slice(c * dc, (c + 1) * dc)
        in_maps.append({
            "xcurT": xcurT,
            "xfullT": xfullT,
            "posT": posT,
            "wq": np.ascontiguousarray(Wq[:, cols]).astype(NPBF16),
            "wk": np.ascontiguousarray(Wkv[:, c * dc:(c + 1) * dc]).astype(NPBF16),
            "wv": np.ascontiguousarray(
                Wkv[:, d + c * dc:d + (c + 1) * dc]).astype(NPBF16),
            "wr": np.ascontiguousarray(Wr[:, cols]).astype(NPBF16),
            "wo": np.ascontiguousarray(Wo[c * dc:(c + 1) * dc, :]).astype(NPBF16),
            "u": np.ascontiguousarray(
                u[c * hpc:(c + 1) * hpc].reshape(dc, 1)).astype(np.float32),
            "v": np.ascontiguousarray(
                v[c * hpc:(c + 1) * hpc].reshape(dc, 1)).astype(np.float32),
        })
    return in_maps


def combine_outputs(results, bo, cs=CS, bs=BS, d=D_MODEL):
    acc = np.zeros((bs, d, cs), np.float32)
    for r in results:
        acc += np.asarray(r["y"], np.float32)
    out = np.transpose(acc, (2, 0, 1))  # (cs, bs, d)
    return (out + np.asarray(bo, np.float32)[None, None, :]).astype(np.float32)


def run(inputs_dict, trace=False):
    dims = (CS, FS, BS, D_MODEL, HPC, HD)
    nc = _get_nc(dims)
    in_maps = make_in_maps(
        inputs_dict["inputs"], inputs_dict["pos_embedding"],
        inputs_dict["full_input"], inputs_dict["u"], inputs_dict["v"],
        inputs_dict["Wkv"], inputs_dict["Wq"], inputs_dict["Wr"],
        inputs_dict["Wo"])
    res = run_bass_kernel_spmd(
        nc, in_maps, core_ids=list(range(N_CORES)), trace=trace)
    out = combine_outputs(res.results, inputs_dict["bo"])
    return out, res


def kernel(**inputs):
    out, _ = run(inputs)
    return out
